# revision 46
# baseline (speedup 1.0000x reference)
"""Multi-Head Latent Attention (naive MLA) on 8 Trainium2 NeuronCores.

Sharding: data-parallel over batch (2) x causal-balanced sequence-parallel
over queries (4-way interleave): core c handles batch b = c//4, query group
g = c%4 (global query rows g, g+4, g+8, ...).  Every core runs the identical
SPMD program; only the data (x shards, wedge-mask matrices) differs.
No collectives: each core produces the full output rows for its queries.

MLA k-absorption: k_h = latent @ Wuk_h is never materialized.  Instead
q'_h = q_h @ Wuk_h^T (tiny 64-contraction matmul) and
scores_h = latent @ q'_h^T, contracting over the latent dim (128) with
latT as the stationary operand.  All operands are bf16 (scores/q'/latent/
v/ctx/Wo); accumulation stays f32 in PSUM.

  latT    = Wdkv^T @ x^T                  [128, 2048]  bf16
  qT      = Wq^T @ xq^T                   [1024, 512]  bf16  (hd-major)
  q'T_h   = Wuk_h^T qT_h                  [128, 16, 512] bf16
  v_aug   = [latent @ Wuv | ones] per key tile           bf16
  scoresT: psum[128 keys, q] = matmul(lhsT=latT tile, rhs=q'T_h); causal
           wedge added by a rank-32 mask matmul; exp on ScalarE over
           two-bank [128,1024] psum tiles (1/sqrt(hd) folded into scale);
           softmax denom comes free as row 64 of the ctx matmul
  ctxT_h  = v_aug^T @ expT                [65, q] psum accum over key tiles
  norm    : reciprocal (DVE) -> partition_broadcast (GPSIMD) -> fused
            multiply-copy psum->sbuf (DVE), bf16
  out     = matmul(lhsT=ctxT tiles, rhs=Wo) -> DMA psum->DRAM f32;
            output projection for q-tile t overlaps attention of t+1.
            bo is added on the host.
"""

import numpy as np

B, S, D, L, H = 2, 2048, 1024, 128, 16
HD = D // H        # 64
AUG = HD + 1       # 65 (v dims + ones column for softmax denominator)
NCORES = 8
GQ = S // 4        # 512 queries per core
QT = 256           # queries per q-tile
NT = GQ // QT      # 2 q-tiles
KT = 128           # keys per key tile
NKT = S // KT      # 16
NEG = -640.0       # additive mask pre-exp-scale (x 1/8 -> -80)
CH = 1024          # exp chunk width (two psum banks)

# single packed [128, PACK_NCOL] bf16 input: column offsets per component
PK_XT = 0                            # xT row-tiled   [128, 8, S]
PK_XQ = PK_XT + (D // 128) * S       # xqT row-tiled  [128, 8, GQ]
PK_WQ = PK_XQ + (D // 128) * GQ      # Wq row-tiled   [128, 8, D]
PK_WDKV = PK_WQ + (D // 128) * D     # Wdkv row-tiled [128, 8, L]
PK_WUK = PK_WDKV + (D // 128) * L    # WukT           [128, 8, L]
PK_WUV = PK_WUK + (H // 2) * L       # Wuv            [128, D]
PK_WO = PK_WUV + D                   # Wo row-tiled   [128, 8, D]
PK_TW = PK_WO + (D // 128) * D       # Twedge         [32, 128]
PK_I32 = PK_TW + 128                 # I32            [32, 32]
PACK_NCOL = PK_I32 + 32

_cache = {}
_DEBUG = False

# schedule knobs (tuned by TimelineSim sweep)
import os as _os
WARM = int(_os.environ.get("K_WARM", "6"))       # PE warmup matmuls
A_RING = _os.environ.get("K_ARING", "pctx")      # psum ring for A groups
A_TRIG = int(_os.environ.get("K_ATRIG", "5"))    # fin_t trigger for A plan
A_RATE = int(_os.environ.get("K_ARATE", "1"))    # A matmuls per iteration
OUT_PACE = int(_os.environ.get("K_OPACE", "2"))  # out_step every N iters


def _use_sel(fill, nw):
    """PSUM hardware constraint: an accumulate (start=False) must directly
    follow its group's start matmul — any intervening start=True in the same
    bank corrupts the accumulation.  A chunk-spanning selector matmul
    therefore can't work; always use the per-strip wedge matmuls."""
    return False


def _worklist(offset):
    """Per q-tile t: list of (u, cs, wedge), identical across cores.

    Query column c of tile t = global row 4*(QT*t+c)+g, position +offset.
    cs (first computed column of the strip) uses the worst core (g=3) so
    strip shapes are core-independent; the wedge matrix (data) carries g.
    """
    work = []
    for t in range(NT):
        items = []
        for u in range(NKT):
            lo = KT * u
            min_qpos = 4 * (QT * t) + 0 + offset
            max_qpos = 4 * (QT * t + QT - 1) + 3 + offset
            if lo + KT - 1 <= min_qpos:
                items.append((u, 0, False))      # fully allowed
            elif lo > max_qpos:
                continue                         # fully masked: skip
            else:
                cs = max(0, -((-(lo - 3 - offset)) // 4) - QT * t)
                assert 0 <= cs < QT
                items.append((u, cs, True))
        assert items and items[0][1] == 0, "first strip must cover col 0"
        work.append(items)
    return work


def _wedge_matrix(g, offset, work):
    """[32, 128] f32: T[m, kj] = NEG where key kj is masked at strip col m.

    Strip col c' (from cs): masked iff kj > 4*c' + r0,
    r0 = 4*(QT*t+cs)+g+offset-lo.  r0 must be tile-independent (asserted)
    so a single matrix serves every partial tile of this core.
    """
    r0s = set()
    for t, items in enumerate(work):
        for (u, cs, wedge) in items:
            if wedge:
                r0s.add(4 * (QT * t + cs) + g + offset - KT * u)
    if not r0s:
        r0s = {g}
    assert len(r0s) == 1, f"non-uniform wedge r0 {r0s} (offset={offset})"
    r0 = r0s.pop()
    assert 0 <= r0 <= 127, r0
    T = np.zeros((32, 128), np.float32)
    for m in range(32):
        T[m, :] = np.where(np.arange(128) > 4 * m + r0, NEG, 0.0)
    return T


def _chunks(items):
    """Split one q-tile's strips for ALL head pairs into bank-aligned
    segments packed densely into one continuous column stream, grouped into
    CH-wide psum chunks.  The stream order is (hp, par, strip): chunks cross
    head-pair boundaries, so every chunk is a uniform CH columns (the totals
    18432/51200 divide exactly) and the exp/scores/ctx pipeline never sees a
    partial chunk or an hp-boundary bubble.  Returns [(fill, [seg])], seg =
    (hp, par, strip_idx, u, cs, s0, s1, o, wedge) with s0/s1 relative to the
    strip's first computed col and o the col offset inside the chunk.
    Segments never cross a 512-col psum bank boundary (matmul outputs can't
    span banks)."""
    segs = []
    o = 0
    for hp in range(H // 2):
        for par in range(2):
            for si, (u, cs, wedge) in enumerate(items):
                sw = QT - cs
                if si == 0 and 512 - (o % 512) < sw:
                    # The si==0 strip opens the ctx accumulation group with
                    # start=True over all columns; splitting it would put two
                    # start=True matmuls in one psum bank with accumulates
                    # that no longer directly follow their group's start --
                    # the hardware corrupts that.  Pad to the next bank.
                    o += 512 - (o % 512)
                s = 0
                while s < sw:
                    room = 512 - (o % 512)
                    w = min(room, sw - s)
                    segs.append((hp, par, si, u, cs, s, s + w, o, wedge))
                    o += w
                    s += w
    total = o
    chunks = []
    for base in range(0, total, CH):
        fill = min(CH, total - base)
        csegs = [(hp, par, si, u, cs, s0, s1, oo - base, wg)
                 for (hp, par, si, u, cs, s0, s1, oo, wg) in segs
                 if base <= oo < base + CH]
        chunks.append((fill, csegs))
    return chunks


def _build(offset):
    import concourse.bacc as bacc
    import concourse.tile as tile
    import concourse.mybir as mybir
    from contextlib import ExitStack

    f32r = mybir.dt.float32r
    bf16 = mybir.dt.bfloat16
    f32 = mybir.dt.float32
    AF = mybir.ActivationFunctionType
    ALU = mybir.AluOpType

    work = _worklist(offset)
    tchunks = [_chunks(work[t]) for t in range(NT)]

    # Host pre-tiles weights to [128 partitions, k-tile, cols] so each tensor
    # loads in ONE DMA with >=1KB contiguous runs (HWDGE issue overhead is
    # ~625ns per DMA instruction, so instruction count matters).  Everything
    # is packed into a SINGLE dram input tensor (one PJRT buffer per core):
    # per-buffer dispatch overhead over axon is ~25-30us, so buffer count
    # dominates the measured end-to-end time.
    nc = bacc.Bacc("TRN2", target_bir_lowering=False, debug=False,
                   num_devices=NCORES)
    inp = nc.dram_tensor("inp", [128, PACK_NCOL], bf16,
                         kind="ExternalInput").ap()
    xT = inp[:, PK_XT:PK_XT + (D // 128) * S]
    xqT = inp[:, PK_XQ:PK_XQ + (D // 128) * GQ]
    Wq = inp[:, PK_WQ:PK_WQ + (D // 128) * D]
    Wdkv = inp[:, PK_WDKV:PK_WDKV + (D // 128) * L]
    WukT = inp[:, PK_WUK:PK_WUK + (H // 2) * L]
    Wuv = inp[:, PK_WUV:PK_WUV + D]
    Wo = inp[:, PK_WO:PK_WO + (D // 128) * D]
    Twedge = inp[0:32, PK_TW:PK_TW + 128]
    I32 = inp[0:32, PK_I32:PK_I32 + 32]
    out = nc.dram_tensor("out", [GQ, D], bf16, kind="ExternalOutput").ap()
    dbg = None
    if _DEBUG:
        dbg = {
            "d_lat": nc.dram_tensor("d_lat", [128, S], mybir.dt.bfloat16,
                                    kind="ExternalOutput").ap(),
            "d_qp": nc.dram_tensor("d_qp", [128, H * GQ], mybir.dt.bfloat16,
                                   kind="ExternalOutput").ap(),
            "d_va": nc.dram_tensor("d_va", [128, NKT * H * AUG],
                                   mybir.dt.bfloat16,
                                   kind="ExternalOutput").ap(),
            "d_ctx": nc.dram_tensor("d_ctx", [128, (H // 2) * GQ],
                                    mybir.dt.bfloat16,
                                    kind="ExternalOutput").ap(),
        }

    with tile.TileContext(nc) as tc, ExitStack() as ctx:
        cp = ctx.enter_context(tc.tile_pool(name="cp", bufs=1, side="right"))
        psc = ctx.enter_context(
            tc.tile_pool(name="psc", bufs=2, space="PSUM", side="left"))
        pctx = ctx.enter_context(
            tc.tile_pool(name="pctx", bufs=3, space="PSUM", side="right"))
        pout = ctx.enter_context(
            tc.tile_pool(name="pout", bufs=1, space="PSUM", side="right"))
        sexp = ctx.enter_context(tc.tile_pool(name="sexp", bufs=4,
                                              side="left"))
        snrm = ctx.enter_context(tc.tile_pool(name="snrm", bufs=3,
                                              side="left"))

        # ---------- input loads (DMA order = need order) ----------
        Wdkv_sb = cp.tile([128, D // 128, L], bf16, tag="Wdkv")
        xT_sb = cp.tile([128, D // 128, S], bf16, tag="xT")
        Wq_sb = cp.tile([128, D // 128, D], bf16, tag="Wq")
        xqT_sb = cp.tile([128, D // 128, GQ], bf16, tag="xqT")
        WukT_sb = cp.tile([128, H // 2, L], bf16, tag="WukT")
        Wuv_sb = cp.tile([128, D], bf16, tag="Wuv")
        tw_sb = cp.tile([32, 128], bf16, tag="tw")
        i32_sb = cp.tile([32, 32], bf16, tag="i32")
        tsel_sb = None
        Wo_sb = cp.tile([128, D // 128, D], bf16, tag="Wo")

        # PE warmup: the cost model (and HW HAM clock gate) runs the PE at
        # 1.2 GHz until ~3us of continuous activity.  Burn the initial DMA
        # wait on dummy matmuls over a memset tile so the first real matmul
        # is already at 2.4 GHz.
        if WARM:
            warm_sb = cp.tile([128, 512], bf16, tag="warm")
            nc.gpsimd.memset(warm_sb[:], 0.0)
            warm_ps = pout.tile([128, 512], f32, tag="o", name="warm")
            for _ in range(WARM):
                nc.tensor.matmul(warm_ps[:], warm_sb[:, 0:128], warm_sb[:],
                                 start=True, stop=True)

        # DMA order = arrival-matches-need for a pipelined start: t=0
        # attention only needs keys 0-1023 (xT blocks 0-1, Wdkv, Wuv), the
        # wedge, and the q path for head pairs 0-1 (xqT, first Wq half,
        # WukT).  xT blocks 2-3 / Wq half 2 / Wo stream in while the first
        # attention chunks run.
        xTr = xT.rearrange("p (a n) -> p a n", n=S)
        Wqr = Wq.rearrange("p (a n) -> p a n", n=D)
        nc.sync.dma_start(Wdkv_sb[:], Wdkv.rearrange("p (a l) -> p a l", l=L))
        nc.sync.dma_start(Wuv_sb[:], Wuv)
        for nb in range(S // 512):
            nc.sync.dma_start(xT_sb[:, :, 512 * nb:512 * (nb + 1)],
                              xTr[:, :, 512 * nb:512 * (nb + 1)])
        nc.sync.dma_start(tw_sb[:], Twedge)
        nc.sync.dma_start(i32_sb[:], I32)
        nc.sync.dma_start(xqT_sb[:], xqT.rearrange("p (a n) -> p a n", n=GQ))
        nc.sync.dma_start(WukT_sb[:], WukT.rearrange("p (a l) -> p a l", l=L))
        nc.sync.dma_start(Wq_sb[:, :, 0:512], Wqr[:, :, 0:512])
        nc.sync.dma_start(Wq_sb[:, :, 512:D], Wqr[:, :, 512:D])
        nc.sync.dma_start(Wo_sb[:], Wo.rearrange("p (a n) -> p a n", n=D))

        copy_engs = [nc.vector.tensor_copy, nc.scalar.copy]

        def copy(i, dst, src):
            copy_engs[i % 2](dst, src)

        # ---------- phase 1+2: latentT; v_aug; qT; q'T ----------
        # All pre-attention.  PSUM ring is deepened to 6 in-flight half-width
        # tiles by borrowing the (still idle) ctx/out pool slots; every
        # [128,512] copy is split across Act and DVE so copy latency hides
        # behind the qT matmuls.
        latTb = cp.tile([128, S], bf16, tag="latTb")
        va_sb = cp.tile([128, NKT, H * AUG], bf16, tag="va")
        qT_sb = cp.tile([128, H // 2, GQ], bf16, tag="qT")
        qpT_sb = cp.tile([128, H, GQ], bf16, tag="qpT")
        # ones column per head slot (softmax denominator lands on psum
        # partition 64); disjoint from the v-dim copies below
        nc.gpsimd.memset(
            va_sb[:].rearrange("p u (h e) -> p u h e", e=AUG)[:, :, :, HD],
            1.0)

        half_pools = [(pctx, "ctx"), (pctx, "ctx"), (pctx, "ctx"),
                      (pout, "o")]
        half_i = [0]

        def half_tile(name):
            pool, tag = half_pools[half_i[0] % len(half_pools)]
            half_i[0] += 1
            return pool.tile([128, 512], f32, tag=tag, name=name)

        def split_copy(i, dst2, src):
            # dst2/src: [128, 2, 256+] pair views — halves to Act and DVE
            copy_engs[i % 2](dst2[0], src[:, 0:256])
            copy_engs[(i + 1) % 2](dst2[1], src[:, 256:512])

        def emit_lat(n):
            ps = psc.tile([128, CH], f32, tag="sc", name=f"lat{n}")
            for h2 in range(2):
                for k in range(D // 128):
                    nc.tensor.matmul(
                        ps[:, 512 * h2:512 * (h2 + 1)], Wdkv_sb[:, k, :],
                        xT_sb[:, k, CH * n + 512 * h2:CH * n + 512 * (h2 + 1)],
                        start=(k == 0), stop=(k == D // 128 - 1))
            nc.vector.tensor_copy(latTb[:, CH * n:CH * n + 512],
                                  ps[:, 0:512])
            nc.scalar.copy(latTb[:, CH * n + 512:CH * (n + 1)],
                           ps[:, 512:CH])

        def emit_va_half(u, half, ring=None):
            if ring is None:
                ps = half_tile(f"v{u}_{half}")
            else:
                ps = ring.tile([128, 512], f32, tag="o", name=f"v{u}_{half}")
            nc.tensor.matmul(ps[:], latTb[:, 128 * u:128 * (u + 1)],
                             Wuv_sb[:, 512 * half:512 * (half + 1)],
                             start=True, stop=True)
            dst = va_sb[:, u, :].rearrange("p (h e) -> p h e", e=AUG)
            src = ps[:].rearrange("p (h e) -> p h e", e=HD)
            copy_engs[(u + half) % 2](
                dst[:, 8 * half:8 * half + 4, 0:HD], src[:, 0:4, :])
            copy_engs[(u + half + 1) % 2](
                dst[:, 8 * half + 4:8 * half + 8, 0:HD], src[:, 4:8, :])

        def emit_qT(mp):
            ps = psc.tile([128, CH], f32, tag="sc", name=f"q{mp}")
            for h2 in range(2):
                m = 2 * mp + h2
                for k in range(D // 128):
                    nc.tensor.matmul(
                        ps[:, 512 * h2:512 * (h2 + 1)],
                        Wq_sb[:, k, 128 * m:128 * (m + 1)], xqT_sb[:, k, :],
                        start=(k == 0), stop=(k == D // 128 - 1))
            nc.vector.tensor_copy(qT_sb[:, 2 * mp, :], ps[:, 0:512])
            nc.scalar.copy(qT_sb[:, 2 * mp + 1, :], ps[:, 512:CH])

        def emit_qp_half(hh, h2, ring=None):
            h = 2 * hh + h2
            hp, p0 = h // 2, 64 * (h % 2)
            if ring is None:
                ps = half_tile(f"qp{h}")
            else:
                ps = ring.tile([128, 512], f32, tag="o", name=f"qp{h}")
            nc.tensor.matmul(ps[:], WukT_sb[p0:p0 + 64, hp, :],
                             qT_sb[p0:p0 + 64, hp, :],
                             start=True, stop=True)
            copy_engs[h % 2](qpT_sb[:, h, 0:256], ps[:, 0:256])
            copy_engs[(h + 1) % 2](qpT_sb[:, h, 256:512], ps[:, 256:512])

        # Prefix: only what attention chunk 0 needs — latT for keys 0-1023
        # (all of t=0), v_aug for those keys, and q' for head pairs 0-1.
        # Everything else streams in through p1q, drained inside the
        # attention loop as its weights arrive (the input DMA stream is
        # bandwidth-serial, so attention starts ~30us earlier than a fully
        # sequential phase 1).
        emit_lat(0)
        emit_lat(1)
        # interleave: big mm-bound qT chunks on the psc ring hide the
        # copy-bound va/qp half chunks rotating the borrowed 4-ring
        va_halves = [(u, h) for u in range(NKT) for h in range(2)]
        qwork = []
        for mp in range(H // 4):
            qwork.append(("qT", mp))
            qwork.append(("qp", 2 * mp, 0))
            qwork.append(("qp", 2 * mp, 1))
            qwork.append(("qp", 2 * mp + 1, 0))
            qwork.append(("qp", 2 * mp + 1, 1))
        vi = 0
        for w in qwork:
            take = 2 if w[0] == "qT" else 1
            for _ in range(take):
                if vi < len(va_halves):
                    emit_va_half(*va_halves[vi])
                    vi += 1
            if w[0] == "qT":
                emit_qT(w[1])
            else:
                emit_qp_half(w[1], w[2])
        while vi < len(va_halves):
            emit_va_half(*va_halves[vi])
            vi += 1

        p1q = []

        def p1_step(budget):
            pass

        # ---------- phase 3+4: attention pipelined with output proj ----------
        ctxT_sb = cp.tile([128, H // 2, GQ], bf16, tag="ctxT")

        flat = []
        for t in range(NT):
            for ci, (fill, segs) in enumerate(tchunks[t]):
                flat.append((t, ci, fill, segs))

        # segs per (t, hp) (and per par0 alone) for ctx-group bookkeeping
        n_items = {}
        n_par0 = {}
        for t in range(NT):
            for (fill, segs) in tchunks[t]:
                for s in segs:
                    key = (t, s[0])
                    n_items[key] = n_items.get(key, 0) + 1
                    if s[1] == 0:
                        n_par0[key] = n_par0.get(key, 0) + 1

        state = {}       # (t, hp) -> [cps, done]
        fin_t = [0] * NT  # finished (t, hp) count
        outq = []        # deferred output-projection emitters

        def emit_scores(idx):
            t, ci, fill, segs = flat[idx]
            sps = psc.tile([128, CH], f32, tag="sc", name=f"s{idx}")
            for (hp, par, si, u, cs, s0, s1, o, wedge) in segs:
                h = 2 * hp + par
                q0 = QT * t + cs
                has_w = wedge and s0 < 32
                nc.tensor.matmul(
                    sps[:, o:o + (s1 - s0)],
                    latTb[:, KT * u:KT * (u + 1)],
                    qpT_sb[:, h, q0 + s0:q0 + s1],
                    start=True, stop=not has_w)
                if has_w:
                    wn = min(32, s1)
                    nc.tensor.matmul(sps[:, o:o + (wn - s0)], tw_sb[:],
                                     i32_sb[:, s0:wn],
                                     start=False, stop=True)
            return sps

        def emit_exp(idx, sps):
            t, ci, fill, segs = flat[idx]
            ex = sexp.tile([128, CH], bf16, tag="exp", name=f"e{idx}")
            nc.scalar.activation(ex[:, 0:fill], sps[:, 0:fill],
                                 AF.Exp, scale=0.125)
            return ex

        def emit_ctx(idx, ex):
            t, ci, fill, segs = flat[idx]
            for (hp, par, si, u, cs, s0, s1, o, wedge) in segs:
                if (t, hp) not in state:
                    cps = pctx.tile([AUG, 2 * QT], f32, tag="ctx",
                                    name=f"c{t}_{hp}")
                    state[(t, hp)] = [cps, 0]
                st = state[(t, hp)]
                cps = st[0]
                h = 2 * hp + par
                st[1] += 1
                nc.tensor.matmul(
                    cps[:, QT * par + cs + s0:QT * par + cs + s1],
                    va_sb[:, u, AUG * h:AUG * (h + 1)],
                    ex[:, o:o + (s1 - s0)],
                    start=(si == 0), stop=(st[1] == n_items[(t, hp)]),
                    skip_group_check=True)
                if st[1] == n_items[(t, hp)]:
                    _finish(t, hp, cps)
                    del state[(t, hp)]

        def _finish(t, hp, cps):
            tq = slice(QT * t, QT * (t + 1))
            # reciprocal straight from psum partition 64 down to an SBUF row
            # on partition 0 (32-aligned cross-base engine access is legal),
            # then GPSIMD broadcasts partition 0 across 64 partitions.
            rcp = snrm.tile([1, 2 * QT], f32r, tag="rcp", name=f"r{t}_{hp}",
                            bufs=3)
            with nc.allow_low_precision(
                    reason="f32r is a bit-identical f32 alias"):
                nc.vector.reciprocal(rcp[:], cps[HD:HD + 1, :])
            rb = snrm.tile([64, 2 * QT], f32r, tag="rb", name=f"b{t}_{hp}",
                           bufs=3)
            nc.gpsimd.partition_broadcast(rb[:], rcp[0:1, :], channels=64)
            nc.vector.tensor_tensor(ctxT_sb[0:HD, hp, tq], cps[0:HD, 0:QT],
                                    rb[:, 0:QT], ALU.mult)
            nc.vector.tensor_tensor(ctxT_sb[HD:128, hp, tq],
                                    cps[0:HD, QT:2 * QT],
                                    rb[:, QT:2 * QT], ALU.mult)
            fin_t[t] += 1
            if fin_t[t] == H // 2 and t != NT - 1:
                # final t's groups are handled by the split tail path
                for m in range(QT // 128):
                    for n in range(D // 512):
                        outq.append((t, m, n))
            if t == NT - 1 and fin_t[t] == A_TRIG:
                a_state["plan"] += [(0, 0), (0, 1), (1, 0), (1, 1)]

        def emit_out(t, m, n):
            ps = pout.tile([128, 512], f32, tag="o", name=f"o{t}_{m}_{n}")
            q0 = QT * t + 128 * m
            for hp in range(H // 2):
                nc.tensor.matmul(ps[:], ctxT_sb[:, hp, q0:q0 + 128],
                                 Wo_sb[:, hp, 512 * n:512 * (n + 1)],
                                 start=(hp == 0), stop=(hp == H // 2 - 1))
            ob = snrm.tile([128, 512], bf16, tag="ob", name=f"ob{t}_{m}_{n}", bufs=2)
            nc.vector.tensor_copy(ob[:], ps[:])
            nc.sync.dma_start(out[q0:q0 + 128, 512 * n:512 * (n + 1)], ob[:])

        # A/B split for the final q-tile's out-projection: the first 7 head
        # pairs' share of the GEMM (the "A" groups) runs in PE gaps during
        # the tail of the last tile's attention, cycling through the pout
        # psum slot and staging to SBUF f32.  The tail then only runs head
        # pair 7's rank-128 update ("B") plus a fused add+cast.  a_step only
        # issues the matmul for head pair hp once hp+2 pairs have finished,
        # so the in-order PE never stalls on an unfinished ctxT row.
        last_t = NT - 1
        a_state = {"plan": [], "obA": {}, "ps": None, "mn": None, "done": 0}

        def a_step(k):
            while k > 0:
                if a_state["ps"] is None:
                    if not a_state["plan"]:
                        return
                    m, n = a_state["plan"][0]
                    if fin_t[last_t] < A_TRIG or outq or out_state:
                        return
                    a_state["mn"] = a_state["plan"].pop(0)
                    a_state["ps"] = (
                        pout.tile([128, 512], f32, tag="o",
                                  name=f"A{m}_{n}") if A_RING == "pout"
                        else pctx.tile([128, 512], f32, tag="ctx",
                                       name=f"A{m}_{n}"))
                    a_state["done"] = 0
                ps = a_state["ps"]
                m, n = a_state["mn"]
                hp = a_state["done"]
                if hp > fin_t[last_t] - 2:
                    return
                q0 = QT * last_t + 128 * m
                nc.tensor.matmul(ps[:], ctxT_sb[:, hp, q0:q0 + 128],
                                 Wo_sb[:, hp, 512 * n:512 * (n + 1)],
                                 start=(hp == 0), stop=(hp == H // 2 - 2))
                a_state["done"] += 1
                k -= 1
                if a_state["done"] == H // 2 - 1:
                    obA = snrm.tile([128, 512], f32, tag=f"obA{m}_{n}",
                                    name=f"obA{m}_{n}", bufs=1)
                    nc.vector.tensor_copy(obA[:], ps[:])
                    a_state["obA"][(m, n)] = obA
                    a_state["ps"] = None

        def emit_out_tail_b(m):
            psb = psc.tile([128, CH], f32, tag="sc", name=f"B{m}")
            q0 = QT * last_t + 128 * m
            for n in range(2):
                nc.tensor.matmul(psb[:, 512 * n:512 * (n + 1)],
                                 ctxT_sb[:, H // 2 - 1, q0:q0 + 128],
                                 Wo_sb[:, H // 2 - 1, 512 * n:512 * (n + 1)],
                                 start=True, stop=True)
            ob = snrm.tile([128, CH], bf16, tag="obt", name=f"obt{m}", bufs=2)
            # GPSIMD cannot read PSUM, so both fused add+casts go on DVE.
            nc.vector.tensor_tensor(ob[:, 0:512], psb[:, 0:512],
                                    a_state["obA"][(m, 0)][:], ALU.add)
            nc.vector.tensor_tensor(ob[:, 512:CH], psb[:, 512:CH],
                                    a_state["obA"][(m, 1)][:], ALU.add)
            nc.sync.dma_start(out[q0:q0 + 128, :], ob[:])

        # Spread each interleaved out-projection group 2 matmuls at a time so
        # PE insertions stay smaller than Act's exp backlog (never starve the
        # softmax stream).
        out_state = []

        def out_step():
            if not out_state:
                if not outq:
                    return
                t, m, n = outq.pop(0)
                ps = pout.tile([128, 512], f32, tag="o", name=f"o{t}_{m}_{n}")
                out_state.append([ps, t, m, n, 0])
            st = out_state[0]
            ps, t, m, n, hp = st
            q0 = QT * t + 128 * m
            for hpp in (hp, hp + 1):
                nc.tensor.matmul(ps[:], ctxT_sb[:, hpp, q0:q0 + 128],
                                 Wo_sb[:, hpp, 512 * n:512 * (n + 1)],
                                 start=(hpp == 0), stop=(hpp == H // 2 - 1))
            st[4] += 2
            if st[4] == H // 2:
                ob = snrm.tile([128, 512], bf16, tag="ob", name=f"ob{t}_{m}_{n}", bufs=2)
                nc.vector.tensor_copy(ob[:], ps[:])
                nc.sync.dma_start(out[q0:q0 + 128, 512 * n:512 * (n + 1)],
                                  ob[:])
                out_state.pop(0)

        pipe_sps = {0: emit_scores(0)}
        pipe_exps = {}
        for i in range(len(flat)):
            if i + 1 < len(flat):
                pipe_sps[i + 1] = emit_scores(i + 1)
            pipe_exps[i] = emit_exp(i, pipe_sps.pop(i))
            if i - 1 >= 0:
                emit_ctx(i - 1, pipe_exps.pop(i - 1))
            if p1q:
                p1_step(3)
            else:
                if i % OUT_PACE == OUT_PACE - 1:
                    out_step()
                a_step(A_RATE)
        last = len(flat) - 1
        # final ctx + finish chain first (DVE/Pool); remaining A groups and
        # the B closes overlap that chain on the PE.
        emit_ctx(last, pipe_exps.pop(last))
        while out_state or outq:
            out_step()
        while a_state["plan"] or a_state["ps"] is not None:
            a_step(7)
        for m in range(QT // 128):
            emit_out_tail_b(m)

        if _DEBUG:
            nc.sync.dma_start(dbg["d_lat"][:], latTb[:])
            nc.sync.dma_start(
                dbg["d_qp"].rearrange("p (h q) -> p h q", q=GQ)[:], qpT_sb[:])
            nc.sync.dma_start(
                dbg["d_va"].rearrange("p (u e) -> p u e", e=H * AUG)[:],
                va_sb[:])
            nc.sync.dma_start(
                dbg["d_ctx"].rearrange("p (a q) -> p a q", q=GQ)[:],
                ctxT_sb[:])

    nc.compile()
    return nc


def _in_maps(x, offset, Wq, Wdkv, Wukv, Wo, bo):
    import ml_dtypes
    bf = ml_dtypes.bfloat16
    work = _worklist(offset)
    # WukT[p, hp, l] = Wukv[l, 64*(2hp + (p>=64)) + p%64]
    Wuk = np.ascontiguousarray(Wukv[:, :D])              # [L, D]
    wukT = Wuk.T.reshape(H, HD, L)                       # [h, hd, L]
    wukT = wukT.reshape(H // 2, 2, HD, L).transpose(1, 2, 0, 3)  # [2,hd,hp,L]
    wukT = np.ascontiguousarray(wukT.reshape(128, (H // 2) * L))
    WoR = np.ascontiguousarray(
        Wo.reshape(D // 128, 128, D).transpose(1, 0, 2).reshape(
            128, (D // 128) * D))
    def ptile(A, cols):   # [D, cols] -> [128, (D//128)*cols] row-tiled
        return np.ascontiguousarray(
            A.reshape(D // 128, 128, cols).transpose(1, 0, 2).reshape(
                128, (D // 128) * cols))

    def pad128(A, cols):  # [p<128, cols] -> [128, cols] zero-padded
        out = np.zeros((128, cols), A.dtype)
        out[:A.shape[0], :A.shape[1]] = A
        return out

    common = {
        "Wq": ptile(np.asarray(Wq), D).astype(bf),
        "Wdkv": ptile(np.asarray(Wdkv), L).astype(bf),
        "WukT": wukT.astype(bf),
        "Wuv": np.ascontiguousarray(Wukv[:, D:]).astype(bf),
        "Wo": WoR.astype(bf),
        "I32": pad128(np.eye(32, dtype=np.float32), 32).astype(bf),
    }
    maps = []
    for c in range(NCORES):
        b, g = c // 4, c % 4
        m = dict(common)
        m["xT"] = ptile(np.ascontiguousarray(x[b].T), S).astype(bf)
        m["xqT"] = ptile(np.ascontiguousarray(x[b, g::4].T), GQ).astype(bf)
        m["Twedge"] = pad128(
            _wedge_matrix(g, offset, work), 128).astype(bf)
        packed = np.concatenate(
            [m["xT"], m["xqT"], m["Wq"], m["Wdkv"], m["WukT"], m["Wuv"],
             m["Wo"], m["Twedge"], m["I32"]], axis=1)
        assert packed.shape == (128, PACK_NCOL), packed.shape
        maps.append({"inp": packed})
    return maps


def kernel(x, offset, Wq, Wdkv, Wukv, Wo, bo):
    from concourse.bass_utils import run_bass_kernel_spmd
    off = int(np.asarray(offset))
    if off not in _cache:
        _cache[off] = _build(off)
    nc = _cache[off]
    maps = _in_maps(np.asarray(x, np.float32), off, Wq, Wdkv, Wukv, Wo, bo)
    res = run_bass_kernel_spmd(nc, maps, list(range(NCORES)))
    outf = np.empty((B, S, D), np.float32)
    for c in range(NCORES):
        b, g = c // 4, c % 4
        outf[b, g::4, :] = np.asarray(res.results[c]["out"], np.float32)
    outf += np.asarray(bo, np.float32)
    return outf



# revision 54
# speedup vs baseline: 1.0029x; 1.0029x over previous
"""Multi-Head Latent Attention (naive MLA) on 8 Trainium2 NeuronCores.

Sharding: data-parallel over batch (2) x causal-balanced sequence-parallel
over queries (4-way interleave): core c handles batch b = c//4, query group
g = c%4 (global query rows g, g+4, g+8, ...).  Every core runs the identical
SPMD program; only the data (x shards, wedge-mask matrices) differs.
No collectives: each core produces the full output rows for its queries.

MLA k-absorption: k_h = latent @ Wuk_h is never materialized.  Instead
q'_h = q_h @ Wuk_h^T (tiny 64-contraction matmul) and
scores_h = latent @ q'_h^T, contracting over the latent dim (128) with
latT as the stationary operand.  All operands are bf16 (scores/q'/latent/
v/ctx/Wo); accumulation stays f32 in PSUM.

  latT    = Wdkv^T @ x^T                  [128, 2048]  bf16
  qT      = Wq^T @ xq^T                   [1024, 512]  bf16  (hd-major)
  q'T_h   = Wuk_h^T qT_h                  [128, 16, 512] bf16
  v_aug   = [latent @ Wuv | ones] per key tile           bf16
  scoresT: psum[128 keys, q] = matmul(lhsT=latT tile, rhs=q'T_h); causal
           wedge added by a rank-32 mask matmul; exp on ScalarE over
           two-bank [128,1024] psum tiles (1/sqrt(hd) folded into scale);
           softmax denom comes free as row 64 of the ctx matmul
  ctxT_h  = v_aug^T @ expT                [65, q] psum accum over key tiles
  norm    : reciprocal (DVE) -> partition_broadcast (GPSIMD) -> fused
            multiply-copy psum->sbuf (DVE), bf16
  out     = matmul(lhsT=ctxT tiles, rhs=Wo) -> copy/add -> bf16 DMA to
            DRAM; bo is added on the host (f32).

Scheduling notes (tuned against TimelineSim; see the K_* env knobs):
  - All inputs ship as ONE packed [128, 40096] bf16 dram tensor and the
    output as one bf16 tensor: per-PJRT-buffer dispatch overhead over axon
    is ~25-30us/buffer, which dominated the old 13-input layout.
  - The per-q-tile score stream packs ALL head pairs' causal strips into
    one continuous column stream of uniform 1024-col chunks (chunks cross
    head-pair boundaries), so the scores->exp->ctx pipeline has no partial
    chunks and no per-head-pair bubbles.  A strip that opens a ctx psum
    accumulation (si==0, start=True over all columns) must never split
    across psum banks -- two start=True matmuls in one bank corrupt the
    group on hardware -- so the stream pads to the next bank when needed.
  - The final q-tile's output projection is split A/B: head pairs 0-6
    accumulate in freed psum slots during the last head pair's attention
    and stage to SBUF; the tail then runs only head pair 7's rank-128
    update plus fused DVE add+casts, with per-half DMAs.
  - A few dummy warmup matmuls at t=0 hold the PE's HAM clock gate at
    2.4 GHz before the first real matmul.
"""

import numpy as np

B, S, D, L, H = 2, 2048, 1024, 128, 16
HD = D // H        # 64
AUG = HD + 1       # 65 (v dims + ones column for softmax denominator)
NCORES = 8
GQ = S // 4        # 512 queries per core
QT = 256           # queries per q-tile
NT = GQ // QT      # 2 q-tiles
KT = 128           # keys per key tile
NKT = S // KT      # 16
NEG = -640.0       # additive mask pre-exp-scale (x 1/8 -> -80)
CH = 1024          # exp chunk width (two psum banks)

# single packed [128, PACK_NCOL] bf16 input: column offsets per component
PK_XT = 0                            # xT row-tiled   [128, 8, S]
PK_XQ = PK_XT + (D // 128) * S       # xqT row-tiled  [128, 8, GQ]
PK_WQ = PK_XQ + (D // 128) * GQ      # Wq row-tiled   [128, 8, D]
PK_WDKV = PK_WQ + (D // 128) * D     # Wdkv row-tiled [128, 8, L]
PK_WUK = PK_WDKV + (D // 128) * L    # WukT           [128, 8, L]
PK_WUV = PK_WUK + (H // 2) * L       # Wuv            [128, D]
PK_WO = PK_WUV + D                   # Wo row-tiled   [128, 8, D]
PK_TW = PK_WO + (D // 128) * D       # Twedge         [32, 128]
PK_I32 = PK_TW + 128                 # I32            [32, 32]
PACK_NCOL = PK_I32 + 32

_cache = {}
_DEBUG = False

# schedule knobs (tuned by TimelineSim sweep)
import os as _os
WARM = int(_os.environ.get("K_WARM", "6"))       # PE warmup matmuls
A_RING = _os.environ.get("K_ARING", "pctx")      # psum ring for A groups
A_TRIG = int(_os.environ.get("K_ATRIG", "5"))    # fin_t trigger for A plan
A_RATE = int(_os.environ.get("K_ARATE", "1"))    # A matmuls per iteration
OUT_PACE = int(_os.environ.get("K_OPACE", "2"))  # out_step every N iters


def _worklist(offset):
    """Per q-tile t: list of (u, cs, wedge), identical across cores.

    Query column c of tile t = global row 4*(QT*t+c)+g, position +offset.
    cs (first computed column of the strip) uses the worst core (g=3) so
    strip shapes are core-independent; the wedge matrix (data) carries g.
    """
    work = []
    for t in range(NT):
        items = []
        for u in range(NKT):
            lo = KT * u
            min_qpos = 4 * (QT * t) + 0 + offset
            max_qpos = 4 * (QT * t + QT - 1) + 3 + offset
            if lo + KT - 1 <= min_qpos:
                items.append((u, 0, False))      # fully allowed
            elif lo > max_qpos:
                continue                         # fully masked: skip
            else:
                cs = max(0, -((-(lo - 3 - offset)) // 4) - QT * t)
                assert 0 <= cs < QT
                items.append((u, cs, True))
        assert items and items[0][1] == 0, "first strip must cover col 0"
        work.append(items)
    return work


def _wedge_matrix(g, offset, work):
    """[32, 128] f32: T[m, kj] = NEG where key kj is masked at strip col m.

    Strip col c' (from cs): masked iff kj > 4*c' + r0,
    r0 = 4*(QT*t+cs)+g+offset-lo.  r0 must be tile-independent (asserted)
    so a single matrix serves every partial tile of this core.
    """
    r0s = set()
    for t, items in enumerate(work):
        for (u, cs, wedge) in items:
            if wedge:
                r0s.add(4 * (QT * t + cs) + g + offset - KT * u)
    if not r0s:
        r0s = {g}
    assert len(r0s) == 1, f"non-uniform wedge r0 {r0s} (offset={offset})"
    r0 = r0s.pop()
    assert 0 <= r0 <= 127, r0
    T = np.zeros((32, 128), np.float32)
    for m in range(32):
        T[m, :] = np.where(np.arange(128) > 4 * m + r0, NEG, 0.0)
    return T


def _chunks(items):
    """Split one q-tile's strips for ALL head pairs into bank-aligned
    segments packed densely into one continuous column stream, grouped into
    CH-wide psum chunks.  The stream order is (hp, par, strip): chunks cross
    head-pair boundaries, so every chunk is a uniform CH columns (the totals
    18432/51200 divide exactly) and the exp/scores/ctx pipeline never sees a
    partial chunk or an hp-boundary bubble.  Returns [(fill, [seg])], seg =
    (hp, par, strip_idx, u, cs, s0, s1, o, wedge) with s0/s1 relative to the
    strip's first computed col and o the col offset inside the chunk.
    Segments never cross a 512-col psum bank boundary (matmul outputs can't
    span banks)."""
    segs = []
    o = 0
    for hp in range(H // 2):
        for par in range(2):
            for si, (u, cs, wedge) in enumerate(items):
                sw = QT - cs
                if si == 0 and 512 - (o % 512) < sw:
                    # The si==0 strip opens the ctx accumulation group with
                    # start=True over all columns; splitting it would put two
                    # start=True matmuls in one psum bank with accumulates
                    # that no longer directly follow their group's start --
                    # the hardware corrupts that.  Pad to the next bank.
                    o += 512 - (o % 512)
                s = 0
                while s < sw:
                    room = 512 - (o % 512)
                    w = min(room, sw - s)
                    segs.append((hp, par, si, u, cs, s, s + w, o, wedge))
                    o += w
                    s += w
    total = o
    chunks = []
    for base in range(0, total, CH):
        fill = min(CH, total - base)
        csegs = [(hp, par, si, u, cs, s0, s1, oo - base, wg)
                 for (hp, par, si, u, cs, s0, s1, oo, wg) in segs
                 if base <= oo < base + CH]
        chunks.append((fill, csegs))
    return chunks


def _build(offset):
    import concourse.bacc as bacc
    import concourse.tile as tile
    import concourse.mybir as mybir
    from contextlib import ExitStack

    f32r = mybir.dt.float32r
    bf16 = mybir.dt.bfloat16
    f32 = mybir.dt.float32
    AF = mybir.ActivationFunctionType
    ALU = mybir.AluOpType

    work = _worklist(offset)
    tchunks = [_chunks(work[t]) for t in range(NT)]

    # Host pre-tiles weights to [128 partitions, k-tile, cols] so each tensor
    # loads in ONE DMA with >=1KB contiguous runs (HWDGE issue overhead is
    # ~625ns per DMA instruction, so instruction count matters).  Everything
    # is packed into a SINGLE dram input tensor (one PJRT buffer per core):
    # per-buffer dispatch overhead over axon is ~25-30us, so buffer count
    # dominates the measured end-to-end time.
    nc = bacc.Bacc("TRN2", target_bir_lowering=False, debug=False,
                   num_devices=NCORES)
    inp = nc.dram_tensor("inp", [128, PACK_NCOL], bf16,
                         kind="ExternalInput").ap()
    xT = inp[:, PK_XT:PK_XT + (D // 128) * S]
    xqT = inp[:, PK_XQ:PK_XQ + (D // 128) * GQ]
    Wq = inp[:, PK_WQ:PK_WQ + (D // 128) * D]
    Wdkv = inp[:, PK_WDKV:PK_WDKV + (D // 128) * L]
    WukT = inp[:, PK_WUK:PK_WUK + (H // 2) * L]
    Wuv = inp[:, PK_WUV:PK_WUV + D]
    Wo = inp[:, PK_WO:PK_WO + (D // 128) * D]
    Twedge = inp[0:32, PK_TW:PK_TW + 128]
    I32 = inp[0:32, PK_I32:PK_I32 + 32]
    out = nc.dram_tensor("out", [GQ, D], bf16, kind="ExternalOutput").ap()
    dbg = None
    if _DEBUG:
        dbg = {
            "d_lat": nc.dram_tensor("d_lat", [128, S], mybir.dt.bfloat16,
                                    kind="ExternalOutput").ap(),
            "d_qp": nc.dram_tensor("d_qp", [128, H * GQ], mybir.dt.bfloat16,
                                   kind="ExternalOutput").ap(),
            "d_va": nc.dram_tensor("d_va", [128, NKT * H * AUG],
                                   mybir.dt.bfloat16,
                                   kind="ExternalOutput").ap(),
            "d_ctx": nc.dram_tensor("d_ctx", [128, (H // 2) * GQ],
                                    mybir.dt.bfloat16,
                                    kind="ExternalOutput").ap(),
        }

    with tile.TileContext(nc) as tc, ExitStack() as ctx:
        cp = ctx.enter_context(tc.tile_pool(name="cp", bufs=1, side="right"))
        psc = ctx.enter_context(
            tc.tile_pool(name="psc", bufs=2, space="PSUM", side="left"))
        pctx = ctx.enter_context(
            tc.tile_pool(name="pctx", bufs=3, space="PSUM", side="right"))
        pout = ctx.enter_context(
            tc.tile_pool(name="pout", bufs=1, space="PSUM", side="right"))
        sexp = ctx.enter_context(tc.tile_pool(name="sexp", bufs=4,
                                              side="left"))
        snrm = ctx.enter_context(tc.tile_pool(name="snrm", bufs=3,
                                              side="left"))

        # ---------- input loads (DMA order = need order) ----------
        Wdkv_sb = cp.tile([128, D // 128, L], bf16, tag="Wdkv")
        xT_sb = cp.tile([128, D // 128, S], bf16, tag="xT")
        Wq_sb = cp.tile([128, D // 128, D], bf16, tag="Wq")
        xqT_sb = cp.tile([128, D // 128, GQ], bf16, tag="xqT")
        WukT_sb = cp.tile([128, H // 2, L], bf16, tag="WukT")
        Wuv_sb = cp.tile([128, D], bf16, tag="Wuv")
        tw_sb = cp.tile([32, 128], bf16, tag="tw")
        i32_sb = cp.tile([32, 32], bf16, tag="i32")
        tsel_sb = None
        Wo_sb = cp.tile([128, D // 128, D], bf16, tag="Wo")

        # PE warmup: the cost model (and HW HAM clock gate) runs the PE at
        # 1.2 GHz until ~3us of continuous activity.  Burn the initial DMA
        # wait on dummy matmuls over a memset tile so the first real matmul
        # is already at 2.4 GHz.
        if WARM:
            warm_sb = cp.tile([128, 512], bf16, tag="warm")
            nc.gpsimd.memset(warm_sb[:], 0.0)
            warm_ps = pout.tile([128, 512], f32, tag="o", name="warm")
            for _ in range(WARM):
                nc.tensor.matmul(warm_ps[:], warm_sb[:, 0:128], warm_sb[:],
                                 start=True, stop=True)

        # DMA order = arrival-matches-need for a pipelined start: t=0
        # attention only needs keys 0-1023 (xT blocks 0-1, Wdkv, Wuv), the
        # wedge, and the q path for head pairs 0-1 (xqT, first Wq half,
        # WukT).  xT blocks 2-3 / Wq half 2 / Wo stream in while the first
        # attention chunks run.
        xTr = xT.rearrange("p (a n) -> p a n", n=S)
        Wqr = Wq.rearrange("p (a n) -> p a n", n=D)
        nc.sync.dma_start(Wdkv_sb[:], Wdkv.rearrange("p (a l) -> p a l", l=L))
        nc.sync.dma_start(Wuv_sb[:], Wuv)
        for nb in range(S // 512):
            nc.sync.dma_start(xT_sb[:, :, 512 * nb:512 * (nb + 1)],
                              xTr[:, :, 512 * nb:512 * (nb + 1)])
        nc.sync.dma_start(tw_sb[:], Twedge)
        nc.sync.dma_start(i32_sb[:], I32)
        nc.sync.dma_start(xqT_sb[:], xqT.rearrange("p (a n) -> p a n", n=GQ))
        nc.sync.dma_start(WukT_sb[:], WukT.rearrange("p (a l) -> p a l", l=L))
        nc.sync.dma_start(Wq_sb[:, :, 0:512], Wqr[:, :, 0:512])
        nc.sync.dma_start(Wq_sb[:, :, 512:D], Wqr[:, :, 512:D])
        nc.sync.dma_start(Wo_sb[:], Wo.rearrange("p (a n) -> p a n", n=D))

        copy_engs = [nc.vector.tensor_copy, nc.scalar.copy]

        def copy(i, dst, src):
            copy_engs[i % 2](dst, src)

        # ---------- phase 1+2: latentT; v_aug; qT; q'T ----------
        # All pre-attention.  PSUM ring is deepened to 6 in-flight half-width
        # tiles by borrowing the (still idle) ctx/out pool slots; every
        # [128,512] copy is split across Act and DVE so copy latency hides
        # behind the qT matmuls.
        latTb = cp.tile([128, S], bf16, tag="latTb")
        va_sb = cp.tile([128, NKT, H * AUG], bf16, tag="va")
        qT_sb = cp.tile([128, H // 2, GQ], bf16, tag="qT")
        qpT_sb = cp.tile([128, H, GQ], bf16, tag="qpT")
        # ones column per head slot (softmax denominator lands on psum
        # partition 64); disjoint from the v-dim copies below
        nc.gpsimd.memset(
            va_sb[:].rearrange("p u (h e) -> p u h e", e=AUG)[:, :, :, HD],
            1.0)

        half_pools = [(pctx, "ctx"), (pctx, "ctx"), (pctx, "ctx"),
                      (pout, "o")]
        half_i = [0]

        def half_tile(name):
            pool, tag = half_pools[half_i[0] % len(half_pools)]
            half_i[0] += 1
            return pool.tile([128, 512], f32, tag=tag, name=name)

        def emit_lat(n):
            ps = psc.tile([128, CH], f32, tag="sc", name=f"lat{n}")
            for h2 in range(2):
                for k in range(D // 128):
                    nc.tensor.matmul(
                        ps[:, 512 * h2:512 * (h2 + 1)], Wdkv_sb[:, k, :],
                        xT_sb[:, k, CH * n + 512 * h2:CH * n + 512 * (h2 + 1)],
                        start=(k == 0), stop=(k == D // 128 - 1))
            nc.vector.tensor_copy(latTb[:, CH * n:CH * n + 512],
                                  ps[:, 0:512])
            nc.scalar.copy(latTb[:, CH * n + 512:CH * (n + 1)],
                           ps[:, 512:CH])

        def emit_va_half(u, half):
            ps = half_tile(f"v{u}_{half}")
            nc.tensor.matmul(ps[:], latTb[:, 128 * u:128 * (u + 1)],
                             Wuv_sb[:, 512 * half:512 * (half + 1)],
                             start=True, stop=True)
            dst = va_sb[:, u, :].rearrange("p (h e) -> p h e", e=AUG)
            src = ps[:].rearrange("p (h e) -> p h e", e=HD)
            copy_engs[(u + half) % 2](
                dst[:, 8 * half:8 * half + 4, 0:HD], src[:, 0:4, :])
            copy_engs[(u + half + 1) % 2](
                dst[:, 8 * half + 4:8 * half + 8, 0:HD], src[:, 4:8, :])

        def emit_qT(mp):
            ps = psc.tile([128, CH], f32, tag="sc", name=f"q{mp}")
            for h2 in range(2):
                m = 2 * mp + h2
                for k in range(D // 128):
                    nc.tensor.matmul(
                        ps[:, 512 * h2:512 * (h2 + 1)],
                        Wq_sb[:, k, 128 * m:128 * (m + 1)], xqT_sb[:, k, :],
                        start=(k == 0), stop=(k == D // 128 - 1))
            nc.vector.tensor_copy(qT_sb[:, 2 * mp, :], ps[:, 0:512])
            nc.scalar.copy(qT_sb[:, 2 * mp + 1, :], ps[:, 512:CH])

        def emit_qp_half(hh, h2):
            h = 2 * hh + h2
            hp, p0 = h // 2, 64 * (h % 2)
            ps = half_tile(f"qp{h}")
            nc.tensor.matmul(ps[:], WukT_sb[p0:p0 + 64, hp, :],
                             qT_sb[p0:p0 + 64, hp, :],
                             start=True, stop=True)
            copy_engs[h % 2](qpT_sb[:, h, 0:256], ps[:, 0:256])
            copy_engs[(h + 1) % 2](qpT_sb[:, h, 256:512], ps[:, 256:512])

        emit_lat(0)
        emit_lat(1)
        # interleave: big mm-bound qT chunks on the psc ring hide the
        # copy-bound va/qp half chunks rotating the borrowed 4-ring
        va_halves = [(u, h) for u in range(NKT) for h in range(2)]
        qwork = []
        for mp in range(H // 4):
            qwork.append(("qT", mp))
            qwork.append(("qp", 2 * mp, 0))
            qwork.append(("qp", 2 * mp, 1))
            qwork.append(("qp", 2 * mp + 1, 0))
            qwork.append(("qp", 2 * mp + 1, 1))
        vi = 0
        for w in qwork:
            take = 2 if w[0] == "qT" else 1
            for _ in range(take):
                if vi < len(va_halves):
                    emit_va_half(*va_halves[vi])
                    vi += 1
            if w[0] == "qT":
                emit_qT(w[1])
            else:
                emit_qp_half(w[1], w[2])
        while vi < len(va_halves):
            emit_va_half(*va_halves[vi])
            vi += 1

        # ---------- phase 3+4: attention pipelined with output proj ----------
        ctxT_sb = cp.tile([128, H // 2, GQ], bf16, tag="ctxT")

        flat = []
        for t in range(NT):
            for ci, (fill, segs) in enumerate(tchunks[t]):
                flat.append((t, ci, fill, segs))

        # segs per (t, hp) (and per par0 alone) for ctx-group bookkeeping
        n_items = {}
        n_par0 = {}
        for t in range(NT):
            for (fill, segs) in tchunks[t]:
                for s in segs:
                    key = (t, s[0])
                    n_items[key] = n_items.get(key, 0) + 1
                    if s[1] == 0:
                        n_par0[key] = n_par0.get(key, 0) + 1

        state = {}       # (t, hp) -> [cps, done]
        fin_t = [0] * NT  # finished (t, hp) count
        outq = []        # deferred output-projection emitters

        def emit_scores(idx):
            t, ci, fill, segs = flat[idx]
            sps = psc.tile([128, CH], f32, tag="sc", name=f"s{idx}")
            for (hp, par, si, u, cs, s0, s1, o, wedge) in segs:
                h = 2 * hp + par
                q0 = QT * t + cs
                has_w = wedge and s0 < 32
                nc.tensor.matmul(
                    sps[:, o:o + (s1 - s0)],
                    latTb[:, KT * u:KT * (u + 1)],
                    qpT_sb[:, h, q0 + s0:q0 + s1],
                    start=True, stop=not has_w)
                if has_w:
                    wn = min(32, s1)
                    nc.tensor.matmul(sps[:, o:o + (wn - s0)], tw_sb[:],
                                     i32_sb[:, s0:wn],
                                     start=False, stop=True)
            return sps

        def emit_exp(idx, sps):
            t, ci, fill, segs = flat[idx]
            ex = sexp.tile([128, CH], bf16, tag="exp", name=f"e{idx}")
            nc.scalar.activation(ex[:, 0:fill], sps[:, 0:fill],
                                 AF.Exp, scale=0.125)
            return ex

        def emit_ctx(idx, ex):
            t, ci, fill, segs = flat[idx]
            for (hp, par, si, u, cs, s0, s1, o, wedge) in segs:
                if (t, hp) not in state:
                    cps = pctx.tile([AUG, 2 * QT], f32, tag="ctx",
                                    name=f"c{t}_{hp}")
                    state[(t, hp)] = [cps, 0]
                st = state[(t, hp)]
                cps = st[0]
                h = 2 * hp + par
                st[1] += 1
                nc.tensor.matmul(
                    cps[:, QT * par + cs + s0:QT * par + cs + s1],
                    va_sb[:, u, AUG * h:AUG * (h + 1)],
                    ex[:, o:o + (s1 - s0)],
                    start=(si == 0), stop=(st[1] == n_items[(t, hp)]),
                    skip_group_check=True)
                if st[1] == n_items[(t, hp)]:
                    _finish(t, hp, cps)
                    del state[(t, hp)]

        def _finish(t, hp, cps):
            tq = slice(QT * t, QT * (t + 1))
            # reciprocal straight from psum partition 64 down to an SBUF row
            # on partition 0 (32-aligned cross-base engine access is legal),
            # then GPSIMD broadcasts partition 0 across 64 partitions.
            rcp = snrm.tile([1, 2 * QT], f32r, tag="rcp", name=f"r{t}_{hp}",
                            bufs=3)
            with nc.allow_low_precision(
                    reason="f32r is a bit-identical f32 alias"):
                nc.vector.reciprocal(rcp[:], cps[HD:HD + 1, :])
            rb = snrm.tile([64, 2 * QT], f32r, tag="rb", name=f"b{t}_{hp}",
                           bufs=3)
            nc.gpsimd.partition_broadcast(rb[:], rcp[0:1, :], channels=64)
            nc.vector.tensor_tensor(ctxT_sb[0:HD, hp, tq], cps[0:HD, 0:QT],
                                    rb[:, 0:QT], ALU.mult)
            nc.vector.tensor_tensor(ctxT_sb[HD:128, hp, tq],
                                    cps[0:HD, QT:2 * QT],
                                    rb[:, QT:2 * QT], ALU.mult)
            fin_t[t] += 1
            if fin_t[t] == H // 2 and t != NT - 1:
                # final t's groups are handled by the split tail path
                for m in range(QT // 128):
                    for n in range(D // 512):
                        outq.append((t, m, n))
            if t == NT - 1 and fin_t[t] == A_TRIG:
                a_state["plan"] += [(0, 0), (0, 1), (1, 0), (1, 1)]

        def emit_out(t, m, n):
            ps = pout.tile([128, 512], f32, tag="o", name=f"o{t}_{m}_{n}")
            q0 = QT * t + 128 * m
            for hp in range(H // 2):
                nc.tensor.matmul(ps[:], ctxT_sb[:, hp, q0:q0 + 128],
                                 Wo_sb[:, hp, 512 * n:512 * (n + 1)],
                                 start=(hp == 0), stop=(hp == H // 2 - 1))
            ob = snrm.tile([128, 512], bf16, tag="ob", name=f"ob{t}_{m}_{n}", bufs=2)
            nc.vector.tensor_copy(ob[:], ps[:])
            nc.sync.dma_start(out[q0:q0 + 128, 512 * n:512 * (n + 1)], ob[:])

        # A/B split for the final q-tile's out-projection: the first 7 head
        # pairs' share of the GEMM (the "A" groups) runs in PE gaps during
        # the tail of the last tile's attention, cycling through the pout
        # psum slot and staging to SBUF f32.  The tail then only runs head
        # pair 7's rank-128 update ("B") plus a fused add+cast.  a_step only
        # issues the matmul for head pair hp once hp+2 pairs have finished,
        # so the in-order PE never stalls on an unfinished ctxT row.
        last_t = NT - 1
        a_state = {"plan": [], "obA": {}, "ps": None, "mn": None, "done": 0}

        def a_step(k):
            while k > 0:
                if a_state["ps"] is None:
                    if not a_state["plan"]:
                        return
                    m, n = a_state["plan"][0]
                    if fin_t[last_t] < A_TRIG or outq or out_state:
                        return
                    a_state["mn"] = a_state["plan"].pop(0)
                    a_state["ps"] = (
                        pout.tile([128, 512], f32, tag="o",
                                  name=f"A{m}_{n}") if A_RING == "pout"
                        else pctx.tile([128, 512], f32, tag="ctx",
                                       name=f"A{m}_{n}"))
                    a_state["done"] = 0
                ps = a_state["ps"]
                m, n = a_state["mn"]
                hp = a_state["done"]
                if hp > fin_t[last_t] - 2:
                    return
                q0 = QT * last_t + 128 * m
                nc.tensor.matmul(ps[:], ctxT_sb[:, hp, q0:q0 + 128],
                                 Wo_sb[:, hp, 512 * n:512 * (n + 1)],
                                 start=(hp == 0), stop=(hp == H // 2 - 2))
                a_state["done"] += 1
                k -= 1
                if a_state["done"] == H // 2 - 1:
                    obA = snrm.tile([128, 512], f32, tag=f"obA{m}_{n}",
                                    name=f"obA{m}_{n}", bufs=1)
                    nc.vector.tensor_copy(obA[:], ps[:])
                    a_state["obA"][(m, n)] = obA
                    a_state["ps"] = None

        def emit_out_tail_b(m):
            psb = psc.tile([128, CH], f32, tag="sc", name=f"B{m}")
            q0 = QT * last_t + 128 * m
            for n in range(2):
                nc.tensor.matmul(psb[:, 512 * n:512 * (n + 1)],
                                 ctxT_sb[:, H // 2 - 1, q0:q0 + 128],
                                 Wo_sb[:, H // 2 - 1, 512 * n:512 * (n + 1)],
                                 start=True, stop=True)
            ob = snrm.tile([128, CH], bf16, tag="obt", name=f"obt{m}", bufs=2)
            # GPSIMD cannot read PSUM, so both fused add+casts go on DVE;
            # each half DMAs out as soon as its add lands.
            nc.vector.tensor_tensor(ob[:, 0:512], psb[:, 0:512],
                                    a_state["obA"][(m, 0)][:], ALU.add)
            nc.sync.dma_start(out[q0:q0 + 128, 0:512], ob[:, 0:512])
            nc.vector.tensor_tensor(ob[:, 512:CH], psb[:, 512:CH],
                                    a_state["obA"][(m, 1)][:], ALU.add)
            nc.sync.dma_start(out[q0:q0 + 128, 512:D], ob[:, 512:CH])

        # Spread each interleaved out-projection group 2 matmuls at a time so
        # PE insertions stay smaller than Act's exp backlog (never starve the
        # softmax stream).
        out_state = []

        def out_step():
            if not out_state:
                if not outq:
                    return
                t, m, n = outq.pop(0)
                ps = pout.tile([128, 512], f32, tag="o", name=f"o{t}_{m}_{n}")
                out_state.append([ps, t, m, n, 0])
            st = out_state[0]
            ps, t, m, n, hp = st
            q0 = QT * t + 128 * m
            for hpp in (hp, hp + 1):
                nc.tensor.matmul(ps[:], ctxT_sb[:, hpp, q0:q0 + 128],
                                 Wo_sb[:, hpp, 512 * n:512 * (n + 1)],
                                 start=(hpp == 0), stop=(hpp == H // 2 - 1))
            st[4] += 2
            if st[4] == H // 2:
                ob = snrm.tile([128, 512], bf16, tag="ob", name=f"ob{t}_{m}_{n}", bufs=2)
                nc.vector.tensor_copy(ob[:], ps[:])
                nc.sync.dma_start(out[q0:q0 + 128, 512 * n:512 * (n + 1)],
                                  ob[:])
                out_state.pop(0)

        pipe_sps = {0: emit_scores(0)}
        pipe_exps = {}
        for i in range(len(flat)):
            if i + 1 < len(flat):
                pipe_sps[i + 1] = emit_scores(i + 1)
            pipe_exps[i] = emit_exp(i, pipe_sps.pop(i))
            if i - 1 >= 0:
                emit_ctx(i - 1, pipe_exps.pop(i - 1))
            if i % OUT_PACE == OUT_PACE - 1:
                out_step()
            a_step(A_RATE)
        last = len(flat) - 1
        # final ctx + finish chain first (DVE/Pool); remaining A groups and
        # the B closes overlap that chain on the PE.
        emit_ctx(last, pipe_exps.pop(last))
        while out_state or outq:
            out_step()
        while a_state["plan"] or a_state["ps"] is not None:
            a_step(7)
        for m in range(QT // 128):
            emit_out_tail_b(m)

        if _DEBUG:
            nc.sync.dma_start(dbg["d_lat"][:], latTb[:])
            nc.sync.dma_start(
                dbg["d_qp"].rearrange("p (h q) -> p h q", q=GQ)[:], qpT_sb[:])
            nc.sync.dma_start(
                dbg["d_va"].rearrange("p (u e) -> p u e", e=H * AUG)[:],
                va_sb[:])
            nc.sync.dma_start(
                dbg["d_ctx"].rearrange("p (a q) -> p a q", q=GQ)[:],
                ctxT_sb[:])

    nc.compile()
    return nc


def _in_maps(x, offset, Wq, Wdkv, Wukv, Wo, bo):
    import ml_dtypes
    bf = ml_dtypes.bfloat16
    work = _worklist(offset)
    # WukT[p, hp, l] = Wukv[l, 64*(2hp + (p>=64)) + p%64]
    Wuk = np.ascontiguousarray(Wukv[:, :D])              # [L, D]
    wukT = Wuk.T.reshape(H, HD, L)                       # [h, hd, L]
    wukT = wukT.reshape(H // 2, 2, HD, L).transpose(1, 2, 0, 3)  # [2,hd,hp,L]
    wukT = np.ascontiguousarray(wukT.reshape(128, (H // 2) * L))
    WoR = np.ascontiguousarray(
        Wo.reshape(D // 128, 128, D).transpose(1, 0, 2).reshape(
            128, (D // 128) * D))
    def ptile(A, cols):   # [D, cols] -> [128, (D//128)*cols] row-tiled
        return np.ascontiguousarray(
            A.reshape(D // 128, 128, cols).transpose(1, 0, 2).reshape(
                128, (D // 128) * cols))

    def pad128(A, cols):  # [p<128, cols] -> [128, cols] zero-padded
        out = np.zeros((128, cols), A.dtype)
        out[:A.shape[0], :A.shape[1]] = A
        return out

    common = {
        "Wq": ptile(np.asarray(Wq), D).astype(bf),
        "Wdkv": ptile(np.asarray(Wdkv), L).astype(bf),
        "WukT": wukT.astype(bf),
        "Wuv": np.ascontiguousarray(Wukv[:, D:]).astype(bf),
        "Wo": WoR.astype(bf),
        "I32": pad128(np.eye(32, dtype=np.float32), 32).astype(bf),
    }
    maps = []
    for c in range(NCORES):
        b, g = c // 4, c % 4
        m = dict(common)
        m["xT"] = ptile(np.ascontiguousarray(x[b].T), S).astype(bf)
        m["xqT"] = ptile(np.ascontiguousarray(x[b, g::4].T), GQ).astype(bf)
        m["Twedge"] = pad128(
            _wedge_matrix(g, offset, work), 128).astype(bf)
        packed = np.concatenate(
            [m["xT"], m["xqT"], m["Wq"], m["Wdkv"], m["WukT"], m["Wuv"],
             m["Wo"], m["Twedge"], m["I32"]], axis=1)
        assert packed.shape == (128, PACK_NCOL), packed.shape
        maps.append({"inp": packed})
    return maps


def kernel(x, offset, Wq, Wdkv, Wukv, Wo, bo):
    from concourse.bass_utils import run_bass_kernel_spmd
    off = int(np.asarray(offset))
    if off not in _cache:
        _cache[off] = _build(off)
    nc = _cache[off]
    maps = _in_maps(np.asarray(x, np.float32), off, Wq, Wdkv, Wukv, Wo, bo)
    res = run_bass_kernel_spmd(nc, maps, list(range(NCORES)))
    outf = np.empty((B, S, D), np.float32)
    for c in range(NCORES):
        b, g = c // 4, c % 4
        outf[b, g::4, :] = np.asarray(res.results[c]["out"], np.float32)
    outf += np.asarray(bo, np.float32)
    return outf

